# revision 40
# baseline (speedup 1.0000x reference)
"""Self-contained Trainium2 Bass kernel for a 2-layer GCN encoder
(PyG GCNConv x2 with LeakyReLU), distributed over 8 NeuronCores.

kernel(**inputs) takes the full unsharded inputs (X [50000,512] f32,
edge_index [2,800000] int64, W1/b1/W2/b2) and returns the full
[50000,128] f32 output. See build() for the device program.
"""

import sys
if "/opt/trn_rl_repo" not in sys.path:
    sys.path.insert(0, "/opt/trn_rl_repo")

import heapq
import math
from dataclasses import dataclass, field

import numpy as np
import ml_dtypes

import concourse.bass as bass
import concourse.tile as tile
from concourse import bacc, mybir
from concourse.bass_utils import run_bass_kernel_spmd

FP32 = mybir.dt.float32
BF16 = mybir.dt.bfloat16
I32 = mybir.dt.int32
I16 = mybir.dt.int16


@dataclass
class Cfg:
    n: int          # real node count
    e: int          # real edge count
    d_in: int
    h1: int
    h2: int
    cores: int = 8
    bpc: int = 50   # 128-node dst blocks per core
    split: int = 32768   # int16 gather index boundary
    grp: int = 4    # dst blocks per dma_gather call
    neg: float = 0.2

    @property
    def npad(self):
        return self.cores * self.bpc * 128

    @property
    def shard(self):
        return self.bpc * 128


@dataclass
class Meta:
    cpa: int  # chunks per block, side A (src < split)
    cpb: int  # chunks per block, side B
    bias1_nz: bool
    bias2_nz: bool
    # per-group idx-tile column offsets: list of (g0, gn, colA, colB)
    groups: list = field(default_factory=list)
    tot_cols: int = 0   # idx tile columns (int16 packed by 16)
    nch: int = 0        # chunks per block total
    blocks: np.ndarray = None   # [cores, bpc] block label per (core, pos)
    grow: np.ndarray = None     # [npad] node -> block*128+lane slot


def _assign(src, dst, npad, nblk, cores, bpc):
    """LPT-balance in-degree across dst blocks, then balance blocks across
    table halves (greedy + swap local search) so per-(block, side) edge
    counts fit the smallest chunk cap. Returns node->(block, lane) and
    block->(core, pos)."""
    hb = bpc // 2
    in_deg = np.bincount(dst, minlength=npad)

    order = np.argsort(-in_deg, kind="stable")
    heap = [(0, b) for b in range(nblk)]
    heapq.heapify(heap)
    counts = np.zeros(nblk, np.int32)
    node_block = np.empty(npad, np.int64)
    node_lane = np.empty(npad, np.int64)
    for v in order:
        s, b = heapq.heappop(heap)
        node_block[v] = b
        node_lane[v] = counts[b]
        counts[b] += 1
        if counts[b] < 128:
            heapq.heappush(heap, (s + int(in_deg[v]), b))

    sb = node_block[src]
    db = node_block[dst]
    M = np.zeros((nblk, nblk), np.int64)
    np.add.at(M, (sb, db), 1)
    M[np.arange(nblk), np.arange(nblk)] += 128  # self loops

    cntA = np.zeros(nblk, np.int64)
    cntB = np.zeros(nblk, np.int64)
    nA = nB = 0
    half = np.empty(nblk, np.int32)
    for b in np.argsort(-M.sum(1), kind="stable"):
        m = M[b]
        if ((cntA + m).max() <= (cntB + m).max() and nA < nblk // 2) \
                or nB >= nblk // 2:
            half[b] = 0; cntA += m; nA += 1
        else:
            half[b] = 1; cntB += m; nB += 1

    rng = np.random.default_rng(0)
    A_list = np.where(half == 0)[0]
    B_list = np.where(half == 1)[0]

    cap = 9 * 128   # target: 9+9 chunks per block

    def score(cA, cB):
        over = np.maximum(cA - cap, 0).sum() + np.maximum(cB - cap, 0).sum()
        at = (cA >= cap - 8).sum() + (cB >= cap - 8).sum()
        return (int(over), int(at))

    cur = score(cntA, cntB)
    for _ in range(300000):
        i = int(rng.integers(len(A_list)))
        j = int(rng.integers(len(B_list)))
        ba, bb = A_list[i], B_list[j]
        dA = M[bb] - M[ba]
        nA2 = cntA + dA
        nB2 = cntB - dA
        ns = score(nA2, nB2)
        if ns <= cur:
            cntA, cntB, cur = nA2, nB2, ns
            A_list[i], B_list[j] = bb, ba
        if cur[0] == 0:
            break

    blk_core = np.empty(nblk, np.int64)
    blk_pos = np.empty(nblk, np.int64)
    for h, lst in ((0, np.sort(A_list)), (1, np.sort(B_list))):
        for c in range(cores):
            sel = lst[c * hb:(c + 1) * hb]
            blk_core[sel] = c
            blk_pos[sel] = h * hb + np.arange(hb)
    return node_block, node_lane, blk_core, blk_pos


def preprocess(cfg: Cfg, X, edge_index, W1, b1, W2, b2):
    """Host-side: shard + edge partitioning. Returns (in_maps, meta)."""
    n, npad, shard = cfg.n, cfg.npad, cfg.shard
    src = np.asarray(edge_index[0], dtype=np.int64)
    dst = np.asarray(edge_index[1], dtype=np.int64)

    deg = np.bincount(dst, minlength=npad).astype(np.float32) + 1.0
    dinv = (1.0 / np.sqrt(deg)).astype(np.float32)

    # self loops for every (padded) node
    asrc = np.concatenate([src, np.arange(npad, dtype=np.int64)])
    adst = np.concatenate([dst, np.arange(npad, dtype=np.int64)])

    # dst blocks are balance-assigned to cores/positions, and each core's
    # shard is split into two position-halves that are allgathered
    # separately (so the second collective can overlap message passing).
    # The g tables hold node n at row perm[n] of table half[n]; gather
    # indices are half-relative.
    hb = cfg.bpc // 2
    nblk = npad // 128
    node_block, node_lane, blk_core, blk_pos = _assign(
        src, dst, npad, nblk, cfg.cores, cfg.bpc)
    grow = node_block * 128 + node_lane          # node -> slot id
    c_ = blk_core[node_block]
    p_ = blk_pos[node_block]
    half_ = (p_ >= hb).astype(np.int64)
    perm = (half_ * (npad // 2) + c_ * (hb * 128) + (p_ - half_ * hb) * 128
            + node_lane)
    asrc = perm[asrc]
    adst = grow[adst]

    blk = adst >> 7
    side = (asrc >= npad // 2).astype(np.int64)
    order = np.lexsort((asrc, side, blk))
    asrc, adst, blk, side = asrc[order], adst[order], blk[order], side[order]

    nblk = npad // 128
    cnt_a = np.bincount(blk[side == 0], minlength=nblk)
    cnt_b = np.bincount(blk[side == 1], minlength=nblk)
    cpa = int(math.ceil(cnt_a.max() / 128)) if cnt_a.max() > 0 else 0
    cpb = int(math.ceil(cnt_b.max() / 128)) if cnt_b.max() > 0 else 0
    cap_a, cap_b = cpa * 128, cpb * 128
    nch = cpa + cpb

    nrows_a = npad // 2
    nrows_b = npad // 2
    spread = (np.arange(max(cap_a, cap_b, 1), dtype=np.int64) * 67)
    idx_a = ((spread[:cap_a] + 97) % nrows_a).astype(np.int16)[None, :] \
        * np.ones((nblk, 1), np.int16) if cap_a else np.zeros((nblk, 1), np.int16)
    idx_a = np.ascontiguousarray(
        ((np.arange(nblk)[:, None] * 997 + spread[None, :cap_a]) % nrows_a
         ).astype(np.int16)) if cap_a else np.zeros((nblk, 1), np.int16)
    idx_b = np.ascontiguousarray(
        ((np.arange(nblk)[:, None] * 997 + spread[None, :cap_b]) % nrows_b
         ).astype(np.int16)) if cap_b else np.zeros((nblk, 1), np.int16)
    assert cfg.bpc % 2 == 0
    dstloc = np.full((nblk, nch * 128), -1, np.int32)

    mask = side == 0
    b_, s_, d_ = blk[mask], asrc[mask], adst[mask]
    start = np.zeros(nblk + 1, np.int64)
    np.cumsum(cnt_a, out=start[1:])
    pos = np.arange(len(b_)) - start[b_]
    idx_a[b_, pos] = s_.astype(np.int16)
    dstloc[b_, pos] = (d_ & 127).astype(np.int32)

    mask = side == 1
    b_, s_, d_ = blk[mask], asrc[mask], adst[mask]
    start = np.zeros(nblk + 1, np.int64)
    np.cumsum(cnt_b, out=start[1:])
    pos = np.arange(len(b_)) - start[b_]
    idx_b[b_, pos] = (s_ - npad // 2).astype(np.int16)
    dstloc[b_, cap_a + pos] = (d_ & 127).astype(np.int32)

    # group layout for gather calls (identical structure on every core)
    groups = []
    col = 0
    for g0 in range(0, cfg.bpc, cfg.grp):
        gn = min(cfg.grp, cfg.bpc - g0)
        col_a = col
        col_b = col + gn * cap_a // 16
        col = col_b + gn * cap_b // 16
        groups.append((g0, gn, col_a, col_b))
    tot_cols = col

    blocks_arr = np.empty((cfg.cores, cfg.bpc), np.int64)
    blocks_arr[blk_core, blk_pos] = np.arange(nblk)
    slot_node = np.empty(npad, np.int64)
    slot_node[grow] = np.arange(npad)

    meta = Meta(
        cpa=cpa, cpb=cpb,
        bias1_nz=bool(np.any(np.asarray(b1) != 0)),
        bias2_nz=bool(np.any(np.asarray(b2) != 0)),
        groups=groups, tot_cols=tot_cols, nch=nch,
        blocks=blocks_arr, grow=grow,
    )

    # replicated tensors
    XT = np.zeros((cfg.d_in, npad), np.float32)
    XT[:, :n] = np.asarray(X, np.float32).T
    XT = XT.astype(ml_dtypes.bfloat16)
    W1b = np.asarray(W1, np.float32).astype(ml_dtypes.bfloat16)
    W2b = np.asarray(W2, np.float32).astype(ml_dtypes.bfloat16)
    iota4 = np.ascontiguousarray(np.broadcast_to(
        np.arange(128, dtype=np.float32)[None, None, :],
        (128, 8, 128))).astype(ml_dtypes.bfloat16)
    dinv_full = np.ascontiguousarray(
        dinv.reshape(nblk, 128).T).astype(np.float32)   # [128, nblk]

    in_maps = []
    for c in range(cfg.cores):
        blocks = list(blocks_arr[c])               # labels in position order
        # flat int16 idx stream in group order: [A segs of group][B segs]
        parts = []
        for (g0, gn, _ca, _cb) in groups:
            bsel = blocks[g0:g0 + gn]
            parts.append(idx_a[bsel, :cap_a].reshape(-1))
            parts.append(idx_b[bsel, :cap_b].reshape(-1))
        flat = np.concatenate(parts) if parts else np.zeros(0, np.int16)
        assert flat.size == tot_cols * 16, (flat.size, tot_cols * 16)
        idx_tile = np.ascontiguousarray(
            np.tile(flat.reshape(-1, 16).T, (8, 1)))          # [128, tot_cols]

        dst_tile = np.ascontiguousarray(
            dstloc[blocks].reshape(cfg.bpc * nch, 128).T)      # [128, bpc*nch]

        node_sel = slot_node[
            (np.asarray(blocks)[:, None] * 128
             + np.arange(128)[None, :]).reshape(-1)]
        dv = dinv[node_sel].reshape(cfg.bpc, 128).T
        m = {
            "xt": np.ascontiguousarray(XT[:, node_sel]),
            "w1": W1b, "w2": W2b,
            "idx": idx_tile,
            "dstloc": dst_tile.astype(np.float32).astype(ml_dtypes.bfloat16),
            "dinv": np.ascontiguousarray(dv).astype(np.float32),
            "dinv08": np.ascontiguousarray(dv * (1.0 - cfg.neg)).astype(np.float32),
            "dinv02": np.ascontiguousarray(dv * cfg.neg).astype(np.float32),
            "iota4": iota4,
        }
        if meta.bias1_nz:
            m["b1bc"] = np.ascontiguousarray(np.broadcast_to(
                np.asarray(b1, np.float32)[None, :], (128, cfg.h1))).astype(np.float32)
        if meta.bias2_nz:
            m["b2bc"] = np.ascontiguousarray(np.broadcast_to(
                np.asarray(b2, np.float32)[None, :], (128, cfg.h2))).astype(np.float32)
        in_maps.append(m)
    return in_maps, meta


def build(cfg: Cfg, meta: Meta, stop_after: str = 'full'):
    nc = bacc.Bacc("TRN2", target_bir_lowering=False, debug=False,
                   num_devices=cfg.cores, num_swdge_queues=4)
    sh, npad = cfg.shard, cfg.npad
    kin, kh1 = cfg.d_in // 128, cfg.h1 // 128
    cpa, cpb, nch = meta.cpa, meta.cpb, meta.nch
    cap_a, cap_b = cpa * 128, cpb * 128
    nrows_h = npad // 2
    hb = cfg.bpc // 2
    AT = mybir.ActivationFunctionType
    OP = mybir.AluOpType

    xt = nc.dram_tensor("xt", [cfg.d_in, sh], BF16, kind="ExternalInput")
    w1 = nc.dram_tensor("w1", [cfg.d_in, cfg.h1], BF16, kind="ExternalInput")
    w2 = nc.dram_tensor("w2", [cfg.h1, cfg.h2], BF16, kind="ExternalInput")
    idx = nc.dram_tensor("idx", [128, meta.tot_cols], I16, kind="ExternalInput")
    dstloc = nc.dram_tensor("dstloc", [128, cfg.bpc * nch], BF16, kind="ExternalInput")
    dinv = nc.dram_tensor("dinv", [128, cfg.bpc], FP32, kind="ExternalInput")
    dinv08 = nc.dram_tensor("dinv08", [128, cfg.bpc], FP32, kind="ExternalInput")
    dinv02 = nc.dram_tensor("dinv02", [128, cfg.bpc], FP32, kind="ExternalInput")
    iota_d = nc.dram_tensor("iota4", [128, 8, 128], BF16, kind="ExternalInput")
    b1bc = (nc.dram_tensor("b1bc", [128, cfg.h1], FP32, kind="ExternalInput")
            if meta.bias1_nz else None)
    b2bc = (nc.dram_tensor("b2bc", [128, cfg.h2], FP32, kind="ExternalInput")
            if meta.bias2_nz else None)
    out = nc.dram_tensor("out", [sh, cfg.h2], FP32, kind="ExternalOutput")

    rg = [list(range(cfg.cores))]
    stop = stop_after

    with tile.TileContext(nc) as tc:
        with (
            tc.tile_pool(name="constp", bufs=1) as constp,
            tc.tile_pool(name="dram", bufs=1, space="DRAM") as dram,
            tc.tile_pool(name="ohp", bufs=8) as ohp,
            tc.tile_pool(name="sp", bufs=4) as sp,
            tc.tile_pool(name="pp", bufs=6, space="PSUM") as pp,
        ):
            g1s0 = dram.tile([sh // 2, cfg.h1], BF16)
            g1s1 = dram.tile([sh // 2, cfg.h1], BF16)
            g1f0 = dram.tile([nrows_h, cfg.h1], BF16, addr_space="Shared")
            g1f1 = dram.tile([nrows_h, cfg.h1], BF16, addr_space="Shared")
            z1d0 = dram.tile([sh // 2, cfg.h1], BF16)
            z1d1 = dram.tile([sh // 2, cfg.h1], BF16)
            g2s0 = dram.tile([sh // 2, cfg.h2], BF16)
            g2s1 = dram.tile([sh // 2, cfg.h2], BF16)
            g2f0 = dram.tile([nrows_h, cfg.h2], BF16, addr_space="Shared")
            g2f1 = dram.tile([nrows_h, cfg.h2], BF16, addr_space="Shared")

            # ---- constants ----
            w1sb = constp.tile([128, kin, cfg.h1], BF16)
            for k in range(kin):
                nc.sync.dma_start(w1sb[:, k, :], w1[k * 128:(k + 1) * 128, :])
            w2sb = constp.tile([128, kh1, cfg.h2], BF16)
            for k in range(kh1):
                nc.sync.dma_start(w2sb[:, k, :], w2[k * 128:(k + 1) * 128, :])
            idxsb = constp.tile([128, meta.tot_cols], I16)
            nc.sync.dma_start(idxsb[:], idx[:])
            dstsb = constp.tile([128, cfg.bpc * nch], BF16)
            nc.sync.dma_start(dstsb[:], dstloc[:])
            dvsb = constp.tile([128, cfg.bpc], FP32)
            nc.sync.dma_start(dvsb[:], dinv[:])
            d08sb = constp.tile([128, cfg.bpc], FP32)
            nc.sync.dma_start(d08sb[:], dinv08[:])
            d02sb = constp.tile([128, cfg.bpc], FP32)
            nc.sync.dma_start(d02sb[:], dinv02[:])
            iotasb = constp.tile([128, 8, 128], BF16)
            nc.sync.dma_start(iotasb[:], iota_d[:])
            b1sb = b2sb = None
            if b1bc is not None:
                b1sb = constp.tile([128, cfg.h1], FP32)
                nc.sync.dma_start(b1sb[:], b1bc[:])
            if b2bc is not None:
                b2sb = constp.tile([128, cfg.h2], FP32)
                nc.sync.dma_start(b2sb[:], b2bc[:])

            # ---- dense phase helper: g = dinv * (inT-tiles @ W) ----
            # processes position-halves [b0, b1); insb columns are relative
            def dense(insb, wsb, kk, h, sink, b0, b1):
                for b in range(b0, b1):
                    rb = b - b0
                    ps = pp.tile([128, h], FP32, tag="ps")
                    for k in range(kk):
                        nc.tensor.matmul(ps[:], insb[:, k, rb * 128:(rb + 1) * 128],
                                         wsb[:, k, :],
                                         start=(k == 0), stop=(k == kk - 1))
                    gt = sp.tile([128, h], BF16, tag="gt")
                    nc.scalar.mul(gt[:], ps[:], dvsb[:, b:b + 1])
                    nc.sync.dma_start(sink[rb * 128:(rb + 1) * 128, :], gt[:])

            qctr = [0, 0]

            def gather_seg(dst_tile, src_ap, nchunks, col0, h, side):
                """dma_gather calls capped at 4KB of output per partition
                (8 chunks at h=256; 16 at h=128). Side A uses queues 0/1,
                side B queues 2/3, so B calls waiting on the second
                allgather never stall A calls."""
                cap = 8  # 1024-idx hard limit per dma_gather
                for off in range(0, nchunks, cap):
                    nck = min(cap, nchunks - off)
                    nc.gpsimd.dma_gather(
                        dst_tile[:, off:off + nck, :], src_ap,
                        idxsb[:, col0 + off * 8: col0 + (off + nck) * 8],
                        nck * 128, nck * 128, h,
                        queue_num=side * 2 + qctr[side] % 2)
                    qctr[side] += 1

            def make_mp(gpa, gpb, gla, glb, h, z_sink, bsb):
                """Returns (issue_a, run). A-side gathers are issued up to
                two groups ahead of compute (and can be pre-issued by the
                caller), so the Q7 only stalls on the B-side allgather
                after useful A work is in flight."""
                groups = meta.groups
                ga_t = {}

                def issue_a(i):
                    if not cpa or i >= len(groups) or i in ga_t:
                        return
                    g0, gn, col_a, col_b = groups[i]
                    t = gpa.tile([128, cfg.grp * cpa, h], BF16, tag="gA")
                    gather_seg(t, gla, gn * cpa, col_a, h, 0)
                    ga_t[i] = t

                def run(hooks=None):
                    hooks = hooks or {}
                    for gi, (g0, gn, col_a, col_b) in enumerate(groups):
                        issue_a(gi)
                        issue_a(gi + 1)
                        issue_a(gi + 2)
                        gb = None
                        if cpb:
                            gb = gpb.tile([128, cfg.grp * cpb, h], BF16,
                                          tag="gB")
                            gather_seg(gb, glb, gn * cpb, col_b, h, 1)
                        if gi in hooks:
                            hooks[gi]()
                        ga = ga_t.pop(gi, None)
                        message_group(g0, gn, ga, gb, h, z_sink, bsb)

                return issue_a, run

            def message_group(g0, gn, ga, gb, h, z_sink, bsb):
                    for j in range(gn):
                        b = g0 + j
                        ps = pp.tile([128, h], FP32, tag="ps")
                        for c0 in range(0, nch, 8):
                            nb = min(8, nch - c0)
                            oh = ohp.tile([128, 8, 128], BF16, tag="oh")
                            dcol = dstsb[:, b * nch + c0:b * nch + c0 + nb]
                            nc.vector.tensor_tensor(
                                oh[:, 0:nb, :], iotasb[:, 0:nb, :],
                                dcol.broadcast_to([128, nb, 128]),
                                op=OP.is_equal)
                            for c in range(c0, c0 + nb):
                                if c < cpa:
                                    mt = ga[:, j * cpa + c, :]
                                else:
                                    mt = gb[:, j * cpb + (c - cpa), :]
                                nc.tensor.matmul(ps[:], oh[:, c - c0, :], mt,
                                                 start=(c == 0),
                                                 stop=(c == nch - 1))
                        if bsb is None:
                            r = sp.tile([128, h], FP32, tag="r")
                            nc.scalar.activation(r[:], ps[:], AT.Relu,
                                                 bias=0.0, scale=d08sb[:, b:b + 1])
                            z_sink(b, ps, d02sb[:, b:b + 1], r)
                        else:
                            t = sp.tile([128, h], FP32, tag="t")
                            nc.vector.tensor_scalar(t[:], ps[:], dvsb[:, b:b + 1],
                                                    None, op0=OP.mult)
                            t2 = sp.tile([128, h], FP32, tag="t2")
                            nc.vector.tensor_tensor(t2[:], t[:], bsb[:], op=OP.add)
                            r = sp.tile([128, h], FP32, tag="r")
                            nc.scalar.activation(r[:], t2[:], AT.Relu,
                                                 bias=0.0, scale=1.0 - cfg.neg)
                            z_sink(b, t2, cfg.neg, r)

            def z1_sink(b, acc, coef, r):
                z = sp.tile([128, cfg.h1], BF16, tag="z1")
                nc.vector.scalar_tensor_tensor(z[:], acc[:], coef, r[:],
                                               op0=OP.mult, op1=OP.add)
                zt, rb = (z1d0, b) if b < hb else (z1d1, b - hb)
                nc.sync.dma_start(zt[rb * 128:(rb + 1) * 128, :], z[:])

            def out_sink(b, acc, coef, r):
                z = sp.tile([128, cfg.h2], FP32, tag="zo")
                nc.vector.scalar_tensor_tensor(z[:], acc[:], coef, r[:],
                                               op0=OP.mult, op1=OP.add)
                nc.sync.dma_start(out[b * 128:(b + 1) * 128, :], z[:])

            # ---- phase 1: g1 shard (two halves, allgathered separately) ----
            with tc.tile_pool(name="xtp", bufs=1) as xtp:
                xt0 = xtp.tile([128, kin, hb * 128], BF16)
                xt1 = xtp.tile([128, kin, sh - hb * 128], BF16)
                for k in range(kin):
                    nc.sync.dma_start(xt0[:, k, :],
                                      xt[k * 128:(k + 1) * 128, 0:hb * 128])
                    nc.sync.dma_start(xt1[:, k, :],
                                      xt[k * 128:(k + 1) * 128, hb * 128:])
                dense(xt0, w1sb, kin, cfg.h1, g1s0, 0, hb)
                if stop != "p1":
                    nc.gpsimd.collective_compute(
                        "AllGather", OP.bypass, replica_groups=rg,
                        ins=[g1s0.opt()], outs=[g1f0.opt()])
                dense(xt1, w1sb, kin, cfg.h1, g1s1, hb, cfg.bpc)
                if stop != "p1":
                    nc.gpsimd.collective_compute(
                        "AllGather", OP.bypass, replica_groups=rg,
                        ins=[g1s1.opt()], outs=[g1f1.opt()])

            # ---- phases 3+4 interleaved: layer-1 message passing with
            # dense2+AG2 for each z1 half emitted as soon as that half's
            # blocks are sunk, so AG2 overlaps mp1's second half instead of
            # running in a dead window after it.
            if stop not in ("p1", "ag1"):
                only = 1 if stop == "p3one" else None
                with tc.tile_pool(name="gp1a", bufs=3) as gp1a, \
                        tc.tile_pool(name="gp1b", bufs=3) as gp1b, \
                        tc.tile_pool(name="gp2a", bufs=4) as gp2a, \
                        tc.tile_pool(name="gp2b", bufs=3) as gp2b, \
                        tc.tile_pool(name="ztp", bufs=2) as ztp:

                    def dense2_part(half, zt, gs):
                        z1t = ztp.tile([128, kh1, sh // 2], BF16, tag="z1t",
                                       name="z1t")
                        for k in range(kh1):
                            nc.sync.dma_start_transpose(
                                out=z1t[:, k, :],
                                in_=zt[:, k * 128:(k + 1) * 128])
                        dense(z1t, w2sb, kh1, cfg.h2, gs,
                              half * hb, half * hb + hb)

                    def ag_part(gs, gf):
                        if stop not in ("p4",):
                            nc.gpsimd.collective_compute(
                                "AllGather", OP.bypass, replica_groups=rg,
                                ins=[gs.opt()], outs=[gf.opt()])

                    def dense2_half(half, zt, gs, gf):
                        dense2_part(half, zt, gs)
                        ag_part(gs, gf)

                    _a1, run1 = make_mp(gp1a, gp1b, g1f0[:, :], g1f1[:, :],
                                        cfg.h1, z1_sink, b1sb)
                    cut = (hb + cfg.grp - 1) // cfg.grp
                    # AG3's trigger is hooked two groups after the dense2
                    # emission: by then the g2s0 writes have landed, so the
                    # Q7 never stalls at the trigger and keeps issuing
                    # gather descriptors.
                    run1(hooks={
                        cut: lambda: dense2_part(0, z1d0, g2s0),
                        cut + 2: lambda: ag_part(g2s0, g2f0)})
                    do_mp2 = stop not in ("p3", "p3one", "p4")
                    if do_mp2:
                        a2, run2 = make_mp(gp2a, gp2b, g2f0[:, :],
                                           g2f1[:, :], cfg.h2, out_sink,
                                           b2sb)
                        a2(0)
                        a2(1)
                        a2(2)
                        a2(3)
                    dense2_half(1, z1d1, g2s1, g2f1)
                    if do_mp2:
                        # ---- phase 6: layer-2 message passing ----
                        run2()

    nc.compile()
    return nc


def install_ntff_hook():
    """The agent image's antenv lacks axon_hooks; graft it so trace=True
    can reach the libaxon_pjrt NTFF profiling C ABI."""
    import sys as _sys, types as _types
    if "antenv.axon_hooks" in _sys.modules:
        return
    _sys.path.insert(0, "/root/.axon_site")
    from trn_agent_boot.trn_boot import _ntff_profile_via_ctypes
    hook = _ntff_profile_via_ctypes("/opt/axon/libaxon_pjrt.so")
    mod = _types.ModuleType("antenv.axon_hooks")
    mod._hook = hook
    mod.get_axon_ntff_profile_hook = lambda: mod._hook
    mod.set_axon_ntff_profile_hook = lambda h: setattr(mod, "_hook", h)
    _sys.modules["antenv.axon_hooks"] = mod
    import antenv
    antenv.axon_hooks = mod


def run(cfg: Cfg, X, edge_index, W1, b1, W2, b2, trace=False,
        stop_after='full', trace_cores=None):
    if trace:
        install_ntff_hook()
    import time
    t0 = time.time()
    in_maps, meta = preprocess(cfg, X, edge_index, W1, b1, W2, b2)
    t1 = time.time()
    nc = build(cfg, meta, stop_after=stop_after)
    t2 = time.time()
    print(f"preprocess {t1-t0:.1f}s, build+compile {t2-t1:.1f}s", flush=True)
    res = run_bass_kernel_spmd(nc, in_maps, core_ids=list(range(cfg.cores)),
                               trace=trace, trace_cores=trace_cores)
    print(f"hw run {time.time()-t2:.1f}s", flush=True)
    fullslots = np.empty((cfg.npad, cfg.h2), np.float32)
    for c in range(cfg.cores):
        o = res.results[c]["out"]
        for p, b in enumerate(meta.blocks[c]):
            fullslots[b * 128:(b + 1) * 128] = o[p * 128:(p + 1) * 128]
    full = fullslots[meta.grow[:cfg.n]]
    return full, res, nc, in_maps, meta


_CFG = Cfg(n=50000, e=800000, d_in=512, h1=256, h2=128,
           cores=8, bpc=50, split=32768, grp=2)


def kernel(X, edge_index, W1, b1, W2, b2):
    full, _res, _nc, _maps, _meta = run(
        _CFG, X, edge_index, W1, b1, W2, b2, trace=False)
    return full



# revision 45
# speedup vs baseline: 1.0138x; 1.0138x over previous
"""Self-contained Trainium2 Bass kernel for a 2-layer GCN encoder
(PyG GCNConv x2 with LeakyReLU), distributed over 8 NeuronCores.

kernel(**inputs) takes the full unsharded inputs (X [50000,512] f32,
edge_index [2,800000] int64, W1/b1/W2/b2) and returns the full
[50000,128] f32 output. See build() for the device program.
"""

import sys
if "/opt/trn_rl_repo" not in sys.path:
    sys.path.insert(0, "/opt/trn_rl_repo")

import heapq
import math
from dataclasses import dataclass, field

import numpy as np
import ml_dtypes

import concourse.bass as bass
import concourse.tile as tile
from concourse import bacc, mybir
from concourse.bass_utils import run_bass_kernel_spmd

FP32 = mybir.dt.float32
BF16 = mybir.dt.bfloat16
I32 = mybir.dt.int32
I16 = mybir.dt.int16


@dataclass
class Cfg:
    n: int          # real node count
    e: int          # real edge count
    d_in: int
    h1: int
    h2: int
    cores: int = 8
    bpc: int = 50   # 128-node dst blocks per core
    split: int = 32768   # int16 gather index boundary
    grp: int = 4    # dst blocks per dma_gather call
    neg: float = 0.2

    @property
    def npad(self):
        return self.cores * self.bpc * 128

    @property
    def shard(self):
        return self.bpc * 128


@dataclass
class Meta:
    cpa: int  # chunks per block, side A (src < split)
    cpb: int  # chunks per block, side B
    bias1_nz: bool
    bias2_nz: bool
    # per-group idx-tile column offsets: list of (g0, gn, colA, colB)
    groups: list = field(default_factory=list)
    tot_cols: int = 0   # idx tile columns (int16 packed by 16)
    nch: int = 0        # chunks per block total
    blocks: np.ndarray = None   # [cores, bpc] block label per (core, pos)
    grow: np.ndarray = None     # [npad] node -> block*128+lane slot


def _assign(src, dst, npad, nblk, cores, bpc):
    """LPT-balance in-degree across dst blocks, then balance blocks across
    table halves (greedy + swap local search) so per-(block, side) edge
    counts fit the smallest chunk cap. Returns node->(block, lane) and
    block->(core, pos)."""
    hb = bpc // 2
    in_deg = np.bincount(dst, minlength=npad)

    order = np.argsort(-in_deg, kind="stable")
    heap = [(0, b) for b in range(nblk)]
    heapq.heapify(heap)
    counts = np.zeros(nblk, np.int32)
    node_block = np.empty(npad, np.int64)
    node_lane = np.empty(npad, np.int64)
    for v in order:
        s, b = heapq.heappop(heap)
        node_block[v] = b
        node_lane[v] = counts[b]
        counts[b] += 1
        if counts[b] < 128:
            heapq.heappush(heap, (s + int(in_deg[v]), b))

    sb = node_block[src]
    db = node_block[dst]
    M = np.zeros((nblk, nblk), np.int64)
    np.add.at(M, (sb, db), 1)
    M[np.arange(nblk), np.arange(nblk)] += 128  # self loops

    cntA = np.zeros(nblk, np.int64)
    cntB = np.zeros(nblk, np.int64)
    nA = nB = 0
    half = np.empty(nblk, np.int32)
    for b in np.argsort(-M.sum(1), kind="stable"):
        m = M[b]
        if ((cntA + m).max() <= (cntB + m).max() and nA < nblk // 2) \
                or nB >= nblk // 2:
            half[b] = 0; cntA += m; nA += 1
        else:
            half[b] = 1; cntB += m; nB += 1

    rng = np.random.default_rng(0)
    A_list = np.where(half == 0)[0]
    B_list = np.where(half == 1)[0]

    cap = 9 * 128   # target: 9+9 chunks per block

    def score(cA, cB):
        over = np.maximum(cA - cap, 0).sum() + np.maximum(cB - cap, 0).sum()
        at = (cA >= cap - 8).sum() + (cB >= cap - 8).sum()
        return (int(over), int(at))

    cur = score(cntA, cntB)
    for _ in range(300000):
        i = int(rng.integers(len(A_list)))
        j = int(rng.integers(len(B_list)))
        ba, bb = A_list[i], B_list[j]
        dA = M[bb] - M[ba]
        nA2 = cntA + dA
        nB2 = cntB - dA
        ns = score(nA2, nB2)
        if ns <= cur:
            cntA, cntB, cur = nA2, nB2, ns
            A_list[i], B_list[j] = bb, ba
        if cur[0] == 0:
            break

    blk_core = np.empty(nblk, np.int64)
    blk_pos = np.empty(nblk, np.int64)
    for h, lst in ((0, np.sort(A_list)), (1, np.sort(B_list))):
        for c in range(cores):
            sel = lst[c * hb:(c + 1) * hb]
            blk_core[sel] = c
            blk_pos[sel] = h * hb + np.arange(hb)
    return node_block, node_lane, blk_core, blk_pos


def preprocess(cfg: Cfg, X, edge_index, W1, b1, W2, b2):
    """Host-side: shard + edge partitioning. Returns (in_maps, meta)."""
    n, npad, shard = cfg.n, cfg.npad, cfg.shard
    src = np.asarray(edge_index[0], dtype=np.int64)
    dst = np.asarray(edge_index[1], dtype=np.int64)

    deg = np.bincount(dst, minlength=npad).astype(np.float32) + 1.0
    dinv = (1.0 / np.sqrt(deg)).astype(np.float32)

    # self loops for every (padded) node
    asrc = np.concatenate([src, np.arange(npad, dtype=np.int64)])
    adst = np.concatenate([dst, np.arange(npad, dtype=np.int64)])

    # dst blocks are balance-assigned to cores/positions, and each core's
    # shard is split into two position-halves that are allgathered
    # separately (so the second collective can overlap message passing).
    # The g tables hold node n at row perm[n] of table half[n]; gather
    # indices are half-relative.
    hb = cfg.bpc // 2
    nblk = npad // 128
    node_block, node_lane, blk_core, blk_pos = _assign(
        src, dst, npad, nblk, cfg.cores, cfg.bpc)
    grow = node_block * 128 + node_lane          # node -> slot id
    c_ = blk_core[node_block]
    p_ = blk_pos[node_block]
    half_ = (p_ >= hb).astype(np.int64)
    perm = (half_ * (npad // 2) + c_ * (hb * 128) + (p_ - half_ * hb) * 128
            + node_lane)
    asrc = perm[asrc]
    adst = grow[adst]

    blk = adst >> 7
    side = (asrc >= npad // 2).astype(np.int64)
    order = np.lexsort((asrc, side, blk))
    asrc, adst, blk, side = asrc[order], adst[order], blk[order], side[order]

    nblk = npad // 128
    cnt_a = np.bincount(blk[side == 0], minlength=nblk)
    cnt_b = np.bincount(blk[side == 1], minlength=nblk)
    cpa = int(math.ceil(cnt_a.max() / 128)) if cnt_a.max() > 0 else 0
    cpb = int(math.ceil(cnt_b.max() / 128)) if cnt_b.max() > 0 else 0
    cap_a, cap_b = cpa * 128, cpb * 128
    nch = cpa + cpb

    nrows_a = npad // 2
    nrows_b = npad // 2
    spread = (np.arange(max(cap_a, cap_b, 1), dtype=np.int64) * 67)
    idx_a = ((spread[:cap_a] + 97) % nrows_a).astype(np.int16)[None, :] \
        * np.ones((nblk, 1), np.int16) if cap_a else np.zeros((nblk, 1), np.int16)
    idx_a = np.ascontiguousarray(
        ((np.arange(nblk)[:, None] * 997 + spread[None, :cap_a]) % nrows_a
         ).astype(np.int16)) if cap_a else np.zeros((nblk, 1), np.int16)
    idx_b = np.ascontiguousarray(
        ((np.arange(nblk)[:, None] * 997 + spread[None, :cap_b]) % nrows_b
         ).astype(np.int16)) if cap_b else np.zeros((nblk, 1), np.int16)
    assert cfg.bpc % 2 == 0
    dstloc = np.full((nblk, nch * 128), -1, np.int32)

    mask = side == 0
    b_, s_, d_ = blk[mask], asrc[mask], adst[mask]
    start = np.zeros(nblk + 1, np.int64)
    np.cumsum(cnt_a, out=start[1:])
    pos = np.arange(len(b_)) - start[b_]
    idx_a[b_, pos] = s_.astype(np.int16)
    dstloc[b_, pos] = (d_ & 127).astype(np.int32)

    mask = side == 1
    b_, s_, d_ = blk[mask], asrc[mask], adst[mask]
    start = np.zeros(nblk + 1, np.int64)
    np.cumsum(cnt_b, out=start[1:])
    pos = np.arange(len(b_)) - start[b_]
    idx_b[b_, pos] = (s_ - npad // 2).astype(np.int16)
    dstloc[b_, cap_a + pos] = (d_ & 127).astype(np.int32)

    # group layout for gather calls (identical structure on every core)
    groups = []
    col = 0
    for g0 in range(0, cfg.bpc, cfg.grp):
        gn = min(cfg.grp, cfg.bpc - g0)
        col_a = col
        col_b = col + gn * cap_a // 16
        col = col_b + gn * cap_b // 16
        groups.append((g0, gn, col_a, col_b))
    tot_cols = col

    blocks_arr = np.empty((cfg.cores, cfg.bpc), np.int64)
    blocks_arr[blk_core, blk_pos] = np.arange(nblk)
    slot_node = np.empty(npad, np.int64)
    slot_node[grow] = np.arange(npad)

    meta = Meta(
        cpa=cpa, cpb=cpb,
        bias1_nz=bool(np.any(np.asarray(b1) != 0)),
        bias2_nz=bool(np.any(np.asarray(b2) != 0)),
        groups=groups, tot_cols=tot_cols, nch=nch,
        blocks=blocks_arr, grow=grow,
    )

    # replicated tensors
    XT = np.zeros((cfg.d_in, npad), np.float32)
    XT[:, :n] = np.asarray(X, np.float32).T
    XT = XT.astype(ml_dtypes.bfloat16)
    W1b = np.asarray(W1, np.float32).astype(ml_dtypes.bfloat16)
    W2b = np.asarray(W2, np.float32).astype(ml_dtypes.bfloat16)
    iota4 = np.ascontiguousarray(np.broadcast_to(
        np.arange(128, dtype=np.float32)[None, None, :],
        (128, 8, 128))).astype(ml_dtypes.bfloat16)
    dinv_full = np.ascontiguousarray(
        dinv.reshape(nblk, 128).T).astype(np.float32)   # [128, nblk]

    in_maps = []
    for c in range(cfg.cores):
        blocks = list(blocks_arr[c])               # labels in position order
        # flat int16 idx stream in group order: [A segs of group][B segs]
        parts = []
        for (g0, gn, _ca, _cb) in groups:
            bsel = blocks[g0:g0 + gn]
            parts.append(idx_a[bsel, :cap_a].reshape(-1))
            parts.append(idx_b[bsel, :cap_b].reshape(-1))
        flat = np.concatenate(parts) if parts else np.zeros(0, np.int16)
        assert flat.size == tot_cols * 16, (flat.size, tot_cols * 16)
        idx_tile = np.ascontiguousarray(
            np.tile(flat.reshape(-1, 16).T, (8, 1)))          # [128, tot_cols]

        dst_tile = np.ascontiguousarray(
            dstloc[blocks].reshape(cfg.bpc * nch, 128).T)      # [128, bpc*nch]

        node_sel = slot_node[
            (np.asarray(blocks)[:, None] * 128
             + np.arange(128)[None, :]).reshape(-1)]
        dv = dinv[node_sel].reshape(cfg.bpc, 128).T
        m = {
            "xt": np.ascontiguousarray(XT[:, node_sel]),
            "w1": W1b, "w2": W2b,
            "idx": idx_tile,
            "dstloc": dst_tile.astype(np.float32).astype(ml_dtypes.bfloat16),
            "dinv": np.ascontiguousarray(dv).astype(np.float32),
            "dinv08": np.ascontiguousarray(dv * (1.0 - cfg.neg)).astype(np.float32),
            "dinv02": np.ascontiguousarray(dv * cfg.neg).astype(np.float32),
            "iota4": iota4,
        }
        if meta.bias1_nz:
            m["b1bc"] = np.ascontiguousarray(np.broadcast_to(
                np.asarray(b1, np.float32)[None, :], (128, cfg.h1))).astype(np.float32)
        if meta.bias2_nz:
            m["b2bc"] = np.ascontiguousarray(np.broadcast_to(
                np.asarray(b2, np.float32)[None, :], (128, cfg.h2))).astype(np.float32)
        in_maps.append(m)
    return in_maps, meta


def build(cfg: Cfg, meta: Meta, stop_after: str = 'full'):
    nc = bacc.Bacc("TRN2", target_bir_lowering=False, debug=False,
                   num_devices=cfg.cores, num_swdge_queues=4)
    sh, npad = cfg.shard, cfg.npad
    kin, kh1 = cfg.d_in // 128, cfg.h1 // 128
    cpa, cpb, nch = meta.cpa, meta.cpb, meta.nch
    cap_a, cap_b = cpa * 128, cpb * 128
    nrows_h = npad // 2
    hb = cfg.bpc // 2
    AT = mybir.ActivationFunctionType
    OP = mybir.AluOpType

    xt = nc.dram_tensor("xt", [cfg.d_in, sh], BF16, kind="ExternalInput")
    w1 = nc.dram_tensor("w1", [cfg.d_in, cfg.h1], BF16, kind="ExternalInput")
    w2 = nc.dram_tensor("w2", [cfg.h1, cfg.h2], BF16, kind="ExternalInput")
    idx = nc.dram_tensor("idx", [128, meta.tot_cols], I16, kind="ExternalInput")
    dstloc = nc.dram_tensor("dstloc", [128, cfg.bpc * nch], BF16, kind="ExternalInput")
    dinv = nc.dram_tensor("dinv", [128, cfg.bpc], FP32, kind="ExternalInput")
    dinv08 = nc.dram_tensor("dinv08", [128, cfg.bpc], FP32, kind="ExternalInput")
    dinv02 = nc.dram_tensor("dinv02", [128, cfg.bpc], FP32, kind="ExternalInput")
    iota_d = nc.dram_tensor("iota4", [128, 8, 128], BF16, kind="ExternalInput")
    b1bc = (nc.dram_tensor("b1bc", [128, cfg.h1], FP32, kind="ExternalInput")
            if meta.bias1_nz else None)
    b2bc = (nc.dram_tensor("b2bc", [128, cfg.h2], FP32, kind="ExternalInput")
            if meta.bias2_nz else None)
    out = nc.dram_tensor("out", [sh, cfg.h2], FP32, kind="ExternalOutput")

    rg = [list(range(cfg.cores))]
    stop = stop_after

    with tile.TileContext(nc) as tc:
        with (
            tc.tile_pool(name="constp", bufs=1) as constp,
            tc.tile_pool(name="dram", bufs=1, space="DRAM") as dram,
            tc.tile_pool(name="ohp", bufs=8) as ohp,
            tc.tile_pool(name="sp", bufs=4) as sp,
            tc.tile_pool(name="pp", bufs=6, space="PSUM") as pp,
        ):
            g1s0 = dram.tile([sh // 2, cfg.h1], BF16)
            g1s1 = dram.tile([sh // 2, cfg.h1], BF16)
            g1f0 = dram.tile([nrows_h, cfg.h1], BF16, addr_space="Shared")
            g1f1 = dram.tile([nrows_h, cfg.h1], BF16, addr_space="Shared")
            z1d0 = dram.tile([sh // 2, cfg.h1], BF16)
            z1d1 = dram.tile([sh // 2, cfg.h1], BF16)
            g2s0 = dram.tile([sh // 2, cfg.h2], BF16)
            g2s1 = dram.tile([sh // 2, cfg.h2], BF16)
            g2f0 = dram.tile([nrows_h, cfg.h2], BF16, addr_space="Shared")
            g2f1 = dram.tile([nrows_h, cfg.h2], BF16, addr_space="Shared")

            # ---- constants ----
            w1sb = constp.tile([128, kin, cfg.h1], BF16)
            for k in range(kin):
                nc.sync.dma_start(w1sb[:, k, :], w1[k * 128:(k + 1) * 128, :])
            w2sb = constp.tile([128, kh1, cfg.h2], BF16)
            for k in range(kh1):
                nc.sync.dma_start(w2sb[:, k, :], w2[k * 128:(k + 1) * 128, :])
            idxsb = constp.tile([128, meta.tot_cols], I16)
            nc.scalar.dma_start(idxsb[:], idx[:])
            dstsb = constp.tile([128, cfg.bpc * nch], BF16)
            nc.sync.dma_start(dstsb[:], dstloc[:])
            dvsb = constp.tile([128, cfg.bpc], FP32)
            nc.sync.dma_start(dvsb[:], dinv[:])
            d08sb = constp.tile([128, cfg.bpc], FP32)
            nc.sync.dma_start(d08sb[:], dinv08[:])
            d02sb = constp.tile([128, cfg.bpc], FP32)
            nc.sync.dma_start(d02sb[:], dinv02[:])
            iotasb = constp.tile([128, 8, 128], BF16)
            nc.sync.dma_start(iotasb[:], iota_d[:])
            b1sb = b2sb = None
            if b1bc is not None:
                b1sb = constp.tile([128, cfg.h1], FP32)
                nc.sync.dma_start(b1sb[:], b1bc[:])
            if b2bc is not None:
                b2sb = constp.tile([128, cfg.h2], FP32)
                nc.sync.dma_start(b2sb[:], b2bc[:])

            # ---- dense phase helper: g = dinv * (inT-tiles @ W) ----
            # processes position-halves [b0, b1); insb columns are relative
            def dense(insb, wsb, kk, h, sink, b0, b1):
                for b in range(b0, b1):
                    rb = b - b0
                    ps = pp.tile([128, h], FP32, tag="ps")
                    for k in range(kk):
                        nc.tensor.matmul(ps[:], insb[:, k, rb * 128:(rb + 1) * 128],
                                         wsb[:, k, :],
                                         start=(k == 0), stop=(k == kk - 1))
                    gt = sp.tile([128, h], BF16, tag="gt")
                    nc.scalar.mul(gt[:], ps[:], dvsb[:, b:b + 1])
                    nc.sync.dma_start(sink[rb * 128:(rb + 1) * 128, :], gt[:])

            qctr = [0, 0]

            def gather_seg(dst_tile, src_ap, nchunks, col0, h, side):
                """dma_gather calls capped at 4KB of output per partition
                (8 chunks at h=256; 16 at h=128). Side A uses queues 0/1,
                side B queues 2/3, so B calls waiting on the second
                allgather never stall A calls."""
                cap = 8  # 1024-idx hard limit per dma_gather
                for off in range(0, nchunks, cap):
                    nck = min(cap, nchunks - off)
                    nc.gpsimd.dma_gather(
                        dst_tile[:, off:off + nck, :], src_ap,
                        idxsb[:, col0 + off * 8: col0 + (off + nck) * 8],
                        nck * 128, nck * 128, h,
                        queue_num=side * 2 + qctr[side] % 2)
                    qctr[side] += 1

            def make_mp(gpa, gpb, gla, glb, h, z_sink, bsb):
                """Returns (issue_a, run). A-side gathers are issued up to
                two groups ahead of compute (and can be pre-issued by the
                caller), so the Q7 only stalls on the B-side allgather
                after useful A work is in flight."""
                groups = meta.groups
                ga_t = {}

                def issue_a(i):
                    if not cpa or i >= len(groups) or i in ga_t:
                        return
                    g0, gn, col_a, col_b = groups[i]
                    t = gpa.tile([128, cfg.grp * cpa, h], BF16, tag="gA")
                    gather_seg(t, gla, gn * cpa, col_a, h, 0)
                    ga_t[i] = t

                def run(hooks=None):
                    hooks = hooks or {}
                    for gi, (g0, gn, col_a, col_b) in enumerate(groups):
                        for k in range(6):
                            issue_a(gi + k)
                        gb = None
                        if cpb:
                            gb = gpb.tile([128, cfg.grp * cpb, h], BF16,
                                          tag="gB")
                            gather_seg(gb, glb, gn * cpb, col_b, h, 1)
                        if gi in hooks:
                            hooks[gi]()
                        ga = ga_t.pop(gi, None)
                        message_group(g0, gn, ga, gb, h, z_sink, bsb)

                return issue_a, run

            def message_group(g0, gn, ga, gb, h, z_sink, bsb):
                    for j in range(gn):
                        b = g0 + j
                        ps = pp.tile([128, h], FP32, tag="ps")
                        for c0 in range(0, nch, 8):
                            nb = min(8, nch - c0)
                            oh = ohp.tile([128, 8, 128], BF16, tag="oh")
                            dcol = dstsb[:, b * nch + c0:b * nch + c0 + nb]
                            nc.vector.tensor_tensor(
                                oh[:, 0:nb, :], iotasb[:, 0:nb, :],
                                dcol.broadcast_to([128, nb, 128]),
                                op=OP.is_equal)
                            for c in range(c0, c0 + nb):
                                if c < cpa:
                                    mt = ga[:, j * cpa + c, :]
                                else:
                                    mt = gb[:, j * cpb + (c - cpa), :]
                                nc.tensor.matmul(ps[:], oh[:, c - c0, :], mt,
                                                 start=(c == 0),
                                                 stop=(c == nch - 1))
                        if bsb is None:
                            r = sp.tile([128, h], FP32, tag="r")
                            nc.scalar.activation(r[:], ps[:], AT.Relu,
                                                 bias=0.0, scale=d08sb[:, b:b + 1])
                            z_sink(b, ps, d02sb[:, b:b + 1], r)
                        else:
                            t = sp.tile([128, h], FP32, tag="t")
                            nc.vector.tensor_scalar(t[:], ps[:], dvsb[:, b:b + 1],
                                                    None, op0=OP.mult)
                            t2 = sp.tile([128, h], FP32, tag="t2")
                            nc.vector.tensor_tensor(t2[:], t[:], bsb[:], op=OP.add)
                            r = sp.tile([128, h], FP32, tag="r")
                            nc.scalar.activation(r[:], t2[:], AT.Relu,
                                                 bias=0.0, scale=1.0 - cfg.neg)
                            z_sink(b, t2, cfg.neg, r)

            def z1_sink(b, acc, coef, r):
                z = sp.tile([128, cfg.h1], BF16, tag="z1")
                nc.vector.scalar_tensor_tensor(z[:], acc[:], coef, r[:],
                                               op0=OP.mult, op1=OP.add)
                zt, rb = (z1d0, b) if b < hb else (z1d1, b - hb)
                nc.sync.dma_start(zt[rb * 128:(rb + 1) * 128, :], z[:])

            def out_sink(b, acc, coef, r):
                z = sp.tile([128, cfg.h2], FP32, tag="zo")
                nc.vector.scalar_tensor_tensor(z[:], acc[:], coef, r[:],
                                               op0=OP.mult, op1=OP.add)
                nc.sync.dma_start(out[b * 128:(b + 1) * 128, :], z[:])

            # ---- phase 1: g1 shard (two halves, allgathered separately) ----
            with tc.tile_pool(name="xtp", bufs=1) as xtp:
                xt0 = xtp.tile([128, kin, hb * 128], BF16)
                xt1 = xtp.tile([128, kin, sh - hb * 128], BF16)
                for k in range(kin):
                    nc.sync.dma_start(xt0[:, k, :],
                                      xt[k * 128:(k + 1) * 128, 0:hb * 128])
                    nc.sync.dma_start(xt1[:, k, :],
                                      xt[k * 128:(k + 1) * 128, hb * 128:])
                dense(xt0, w1sb, kin, cfg.h1, g1s0, 0, hb)
                if stop != "p1":
                    nc.gpsimd.collective_compute(
                        "AllGather", OP.bypass, replica_groups=rg,
                        ins=[g1s0.opt()], outs=[g1f0.opt()])
                dense(xt1, w1sb, kin, cfg.h1, g1s1, hb, cfg.bpc)
                if stop != "p1":
                    nc.gpsimd.collective_compute(
                        "AllGather", OP.bypass, replica_groups=rg,
                        ins=[g1s1.opt()], outs=[g1f1.opt()])

            # ---- phases 3+4 interleaved: layer-1 message passing with
            # dense2+AG2 for each z1 half emitted as soon as that half's
            # blocks are sunk, so AG2 overlaps mp1's second half instead of
            # running in a dead window after it.
            if stop not in ("p1", "ag1"):
                only = 1 if stop == "p3one" else None
                with tc.tile_pool(name="gp1a", bufs=6) as gp1a, \
                        tc.tile_pool(name="gp1b", bufs=3) as gp1b, \
                        tc.tile_pool(name="gp2a", bufs=6) as gp2a, \
                        tc.tile_pool(name="gp2b", bufs=3) as gp2b, \
                        tc.tile_pool(name="ztp", bufs=2) as ztp:

                    def dense2_half(half, zt, gs, gf):
                        z1t = ztp.tile([128, kh1, sh // 2], BF16, tag="z1t",
                                       name="z1t")
                        for k in range(kh1):
                            nc.sync.dma_start_transpose(
                                out=z1t[:, k, :],
                                in_=zt[:, k * 128:(k + 1) * 128])
                        dense(z1t, w2sb, kh1, cfg.h2, gs,
                              half * hb, half * hb + hb)
                        if stop not in ("p4",):
                            nc.gpsimd.collective_compute(
                                "AllGather", OP.bypass, replica_groups=rg,
                                ins=[gs.opt()], outs=[gf.opt()])

                    _a1, run1 = make_mp(gp1a, gp1b, g1f0[:, :], g1f1[:, :],
                                        cfg.h1, z1_sink, b1sb)
                    cut = (hb + cfg.grp - 1) // cfg.grp
                    run1(hooks={
                        cut: lambda: dense2_half(0, z1d0, g2s0, g2f0)})
                    do_mp2 = stop not in ("p3", "p3one", "p4")
                    if do_mp2:
                        a2, run2 = make_mp(gp2a, gp2b, g2f0[:, :],
                                           g2f1[:, :], cfg.h2, out_sink,
                                           b2sb)
                        for k in range(6):
                            a2(k)
                    dense2_half(1, z1d1, g2s1, g2f1)
                    if do_mp2:
                        # ---- phase 6: layer-2 message passing ----
                        run2()

    nc.compile()
    return nc


def install_ntff_hook():
    """The agent image's antenv lacks axon_hooks; graft it so trace=True
    can reach the libaxon_pjrt NTFF profiling C ABI."""
    import sys as _sys, types as _types
    if "antenv.axon_hooks" in _sys.modules:
        return
    _sys.path.insert(0, "/root/.axon_site")
    from trn_agent_boot.trn_boot import _ntff_profile_via_ctypes
    hook = _ntff_profile_via_ctypes("/opt/axon/libaxon_pjrt.so")
    mod = _types.ModuleType("antenv.axon_hooks")
    mod._hook = hook
    mod.get_axon_ntff_profile_hook = lambda: mod._hook
    mod.set_axon_ntff_profile_hook = lambda h: setattr(mod, "_hook", h)
    _sys.modules["antenv.axon_hooks"] = mod
    import antenv
    antenv.axon_hooks = mod


def run(cfg: Cfg, X, edge_index, W1, b1, W2, b2, trace=False,
        stop_after='full', trace_cores=None):
    if trace:
        install_ntff_hook()
    import time
    t0 = time.time()
    in_maps, meta = preprocess(cfg, X, edge_index, W1, b1, W2, b2)
    t1 = time.time()
    nc = build(cfg, meta, stop_after=stop_after)
    t2 = time.time()
    print(f"preprocess {t1-t0:.1f}s, build+compile {t2-t1:.1f}s", flush=True)
    res = run_bass_kernel_spmd(nc, in_maps, core_ids=list(range(cfg.cores)),
                               trace=trace, trace_cores=trace_cores)
    print(f"hw run {time.time()-t2:.1f}s", flush=True)
    fullslots = np.empty((cfg.npad, cfg.h2), np.float32)
    for c in range(cfg.cores):
        o = res.results[c]["out"]
        for p, b in enumerate(meta.blocks[c]):
            fullslots[b * 128:(b + 1) * 128] = o[p * 128:(p + 1) * 128]
    full = fullslots[meta.grow[:cfg.n]]
    return full, res, nc, in_maps, meta


_CFG = Cfg(n=50000, e=800000, d_in=512, h1=256, h2=128,
           cores=8, bpc=50, split=32768, grp=2)


def kernel(X, edge_index, W1, b1, W2, b2):
    full, _res, _nc, _maps, _meta = run(
        _CFG, X, edge_index, W1, b1, W2, b2, trace=False)
    return full

